# revision 10
# baseline (speedup 1.0000x reference)
"""Masked 5x5 conv (PixelCNN 'A' mask) on 8 Trainium2 NeuronCores.

Problem (hardcoded): x[4,192,128,128] f32, weight[384,192,5,5] f32,
bias[384] f32, mask[4,1,128,128] i32.
out = where(window_any(mask), conv(x, weight*maskA) + bias, 0).

The 'A' causal mask keeps 12 of 25 taps: rows kh=0,1 fully, row kh=2 only
kw=0,1 -- i.e. every tap reads the current output row or rows above it.

Sharding: core c = (batch b = c//2, row-half = c%2). Each core computes one
batch's 64 output rows for all 384 out channels (3 M=128 chunks).

Per output tile [128 cout, 4 rows x 128 cols = 512] we accumulate 18 K=128
bf16 matmuls into one PSUM bank:
  - 12 taps x channel-chunk ci[0:128]  (from tile xa)
  - 5 tap-PAIRS x ci[128:192]          (from tile xb: lower 64 partitions =
    ci[128:192] data, upper 64 = same data shifted 1 col, so one K=128
    matmul covers two taps that differ by (0,+1))
  - 1 tap-pair (0,4)+(1,4) x ci[128:192] (tile xc: upper shifted one row)
Epilogue: one DVE scalar_tensor_tensor: out = (psum + bias) * valid.
"""

import numpy as np
import ml_dtypes

import concourse.bass as bass
from concourse import mybir
from concourse.bass_utils import run_bass_kernel_spmd

B, CIN, COUT, H, W = 4, 192, 384, 128, 128
KH = KW = 5
PAD = 2
NCORES = 8
HHALF = 64          # output rows per core
NROWS = HHALF + 2   # input rows staged per core (2 above)
WP = W + 4          # padded width
FLAT = NROWS * WP   # 66*132 = 8712
RB = 4              # output rows per block
NBLK = HHALF // RB  # 16 blocks
NFREE = RB * W      # 512 = one PSUM bank of fp32

# Active taps of the 'A' mask, (kh, kw)
TAPS = [(0, 0), (0, 1), (0, 2), (0, 3), (0, 4),
        (1, 0), (1, 1), (1, 2), (1, 3), (1, 4),
        (2, 0), (2, 1)]
# ci[128:192] handled as pairs packed into K=128 matmuls.
# slab xb (upper shifted +1 element = +1 col): pairs differing by (0,1)
PAIRS_XB = [((0, 0), (0, 1)), ((0, 2), (0, 3)),
            ((1, 0), (1, 1)), ((1, 2), (1, 3)), ((2, 0), (2, 1))]
# slab xc (upper shifted +132 elements = +1 row): the leftover pair
PAIR_XC = ((0, 4), (1, 4))

BF16 = ml_dtypes.bfloat16

WSLOT = 18 * 128    # weight cols per m-chunk (m-major layout: [m][s][128])


def _build_program():
    """Raw Bass (no Tile): this walrus build rejects instructions carrying
    more than ~1 embedded sync wait, so all synchronization is standalone
    wait_ge instructions with manually-managed semaphores.

    Schedule (per core):
      - DMA cost model (HW-measured): ~0.65us serial issue per dma_start on
        the issuing engine + ~2us fixed latency + aggregate ~350GB/s.  So:
        few, large DMAs, issued from BOTH the Sync and Scalar HWDGE rings in
        parallel, in deadline order, with no wait_ge serialization between
        input waves (ring FIFO + issue order gives the priority).
      - Weights are laid out m-major ([m][slot][128]) so the first wave only
        carries the 12 slots of m=0 (393KB) instead of all m (1.18MB).
      - PE pre-warm: ~10 dummy matmuls during the initial DMA wait flip the
        HAM clock gate to 2.4 GHz just as the first real data lands, so the
        real stream starts warm and never idles >3.4us.
      - Phase A runs the 12 xa-slots of tiles 0..7 as soon as the first
        weight/xa chunks land; phase B completes those tiles with the
        xb/xc pair slots once those tensors arrive; then steady state:
        18 K=128 matmuls per [128 cout x 512 spatial] PSUM tile.
      - DVE fuses (psum + bias) * valid into one scalar_tensor_tensor per
        tile, writing a bf16 staging buffer; outputs stream out in 2-tile
        chunks; the final tile's two half-tile DMAs are split across the
        Sync and Scalar rings to shorten the tail."""
    nc = bass.Bass()
    bf = mybir.dt.bfloat16
    f32 = mybir.dt.float32

    xa_d = nc.dram_tensor("xa", [128, FLAT], bf, kind="ExternalInput")
    xb_d = nc.dram_tensor("xb", [128, FLAT], bf, kind="ExternalInput")
    xc_d = nc.dram_tensor("xc", [128, FLAT], bf, kind="ExternalInput")
    wt_d = nc.dram_tensor("wt", [128, 3 * WSLOT], bf, kind="ExternalInput")
    bt_d = nc.dram_tensor("bt", [128, 3], f32, kind="ExternalInput")
    vt_d = nc.dram_tensor("vt", [128, HHALF * W], bf, kind="ExternalInput")
    out_d = nc.dram_tensor("out", [128, 3 * HHALF * W], bf, kind="ExternalOutput")

    NPS = 8           # psum banks in rotation
    PHA = 8           # tiles 0..PHA-1 run split-phase (xa first, xb/xc later)
    XA0 = 14 * WP     # xa rows 0..13: blocks 0..2
    XAm = 26 * WP     # xa wave-1b split point (sync half: rows 14..25)
    XA1 = 38 * WP     # rows 0..37 cover output blocks 0..8
    XB1h = 19 * WP    # xb/xc first-wave split point
    XAr2 = 52 * WP    # rest split point (rows 38..51 / 52..65)
    OCH = 2           # out-DMA granularity: blocks per chunk
    NT = 3 * NBLK     # 48 tiles
    VH = HHALF * W // 2

    from contextlib import ExitStack
    with ExitStack() as ctx:
        xa_t = ctx.enter_context(nc.sbuf_tensor([128, FLAT], bf))
        xb_t = ctx.enter_context(nc.sbuf_tensor([128, FLAT], bf))
        xc_t = ctx.enter_context(nc.sbuf_tensor([128, FLAT], bf))
        wt_t = ctx.enter_context(nc.sbuf_tensor([128, 3 * WSLOT], bf))
        bt_t = ctx.enter_context(nc.sbuf_tensor([128, 3], f32))
        vt_t = ctx.enter_context(nc.sbuf_tensor([128, HHALF * W], bf))
        st_t = ctx.enter_context(nc.sbuf_tensor([128, 3 * HHALF * W], bf))
        ps_t = ctx.enter_context(nc.psum_tensor([128, NPS * NFREE], f32))
        da0 = ctx.enter_context(nc.semaphore("da0"))
        da1 = ctx.enter_context(nc.semaphore("da1"))
        da2 = ctx.enter_context(nc.semaphore("da2"))
        db1 = ctx.enter_context(nc.semaphore("db1"))
        db2 = ctx.enter_context(nc.semaphore("db2"))
        dc1 = ctx.enter_context(nc.semaphore("dc1"))
        dc2 = ctx.enter_context(nc.semaphore("dc2"))
        dwt1 = ctx.enter_context(nc.semaphore("dwt1"))
        dwt2 = ctx.enter_context(nc.semaphore("dwt2"))
        dwt3 = ctx.enter_context(nc.semaphore("dwt3"))
        drest = ctx.enter_context(nc.semaphore("drest"))
        pes = ctx.enter_context(nc.semaphore("pes"))
        dve = ctx.enter_context(nc.semaphore("dve"))
        dout = ctx.enter_context(nc.semaphore("dout"))
        warm = ctx.enter_context(nc.semaphore("warm"))
        block = ctx.enter_context(nc.Block())
        xa_v = xa_t[:].rearrange("p (r c) -> p r c", c=WP)
        xb_v = xb_t[:].rearrange("p (r c) -> p r c", c=WP)
        xc_v = xc_t[:].rearrange("p (r c) -> p r c", c=WP)

        # (global weight-slot index, view, kh, kw)
        slots_a = [(s, xa_v, kh, kw) for s, (kh, kw) in enumerate(TAPS)]
        slots_bc = [(12 + i, xb_v, ta[0], ta[1])
                    for i, (ta, _tb) in enumerate(PAIRS_XB)]
        slots_bc += [(17, xc_v, PAIR_XC[0][0], PAIR_XC[0][1])]

        def wsl(s, m):
            return wt_t[:, m * WSLOT + s * 128: m * WSLOT + (s + 1) * 128]

        def emit_mms(tensor, k, sl, start, stop):
            m, blk = divmod(k, NBLK)
            j0 = blk * RB
            ps = ps_t[:, (k % NPS) * NFREE:(k % NPS + 1) * NFREE]
            n = len(sl)
            for i, (s, view, kh, kw) in enumerate(sl):
                mm = nc.tensor.matmul(
                    ps,
                    wsl(s, m),
                    view[:, j0 + kh: j0 + kh + RB, kw: kw + W],
                    start=(start and i == 0),
                    stop=(stop and i == n - 1),
                )
                if stop and i == n - 1:
                    mm.then_inc(pes, 1)

        @block.sync
        def _(sync):
            # input issues, deadline-ordered, no inter-wave waits (ring FIFO
            # + issue order is the priority mechanism). The first micro-chunk
            # (slots 0-1 of m=0 + xa rows 0..5 on the scalar ring) lets the
            # real MM stream start ~1.5us before the rest of wave 1 lands.
            sync.dma_start(wt_t[:, 0:2 * 128], wt_d[:, 0:2 * 128]).then_inc(dwt1, 16)
            sync.dma_start(wt_t[:, 2 * 128:12 * 128], wt_d[:, 2 * 128:12 * 128]).then_inc(dwt1, 16)
            sync.dma_start(xa_t[:, XA0:XAm], xa_d[:, XA0:XAm]).then_inc(da1, 16)
            sync.dma_start(xb_t[:, 0:XB1h], xb_d[:, 0:XB1h]).then_inc(db1, 16)
            sync.dma_start(xc_t[:, 0:XB1h], xc_d[:, 0:XB1h]).then_inc(dc1, 16)
            sync.dma_start(wt_t[:, 12 * 128:WSLOT], wt_d[:, 12 * 128:WSLOT]).then_inc(dwt2, 16)
            sync.dma_start(vt_t[:, 0:VH], vt_d[:, 0:VH]).then_inc(drest, 16)
            sync.dma_start(xa_t[:, XA1:XAr2], xa_d[:, XA1:XAr2]).then_inc(da2, 16)
            sync.dma_start(xb_t[:, XA1:XAr2], xb_d[:, XA1:XAr2]).then_inc(db2, 16)
            sync.dma_start(xc_t[:, XA1:XAr2], xc_d[:, XA1:XAr2]).then_inc(dc2, 16)
            # output chunks of OCH tiles; the final tile's two halves are
            # co-issued with the Scalar ring (see scalar block) so the last
            # DMAs overlap instead of serializing on one engine
            nch = NT // OCH
            ninc = 0
            for c in range(nch):
                lo, hi = c * OCH * NFREE, (c + 1) * OCH * NFREE
                if c == nch - 1:
                    # tile 46, then quarter 3 of tile 47 (half 1 and quarter
                    # 4 go out on the scalar ring; the last DVE unit is a
                    # quarter so the final gating chain is as short as
                    # possible)
                    sync.wait_ge(dve, NT - 1)
                    mid = lo + NFREE
                    sync.dma_start(out_d[:, lo:mid], st_t[:, lo:mid]).then_inc(dout, 16)
                    mid2 = mid + NFREE // 2
                    mid3 = mid2 + NFREE // 4
                    sync.wait_ge(dve, NT + 1)
                    sync.dma_start(out_d[:, mid2:mid3], st_t[:, mid2:mid3]).then_inc(dout, 16)
                    ninc += 3
                else:
                    sync.wait_ge(dve, OCH * (c + 1))
                    sync.dma_start(out_d[:, lo:hi], st_t[:, lo:hi]).then_inc(dout, 16)
                    ninc += 1
            sync.wait_ge(dout, 16 * ninc)

        @block.scalar
        def _(scalar):
            scalar.dma_start(xa_t[:, 0:6 * WP], xa_d[:, 0:6 * WP]).then_inc(da0, 16)
            scalar.dma_start(xa_t[:, 6 * WP:XA0], xa_d[:, 6 * WP:XA0]).then_inc(da0, 16)
            scalar.dma_start(xa_t[:, XAm:XA1], xa_d[:, XAm:XA1]).then_inc(da1, 16)
            scalar.dma_start(xb_t[:, XB1h:XA1], xb_d[:, XB1h:XA1]).then_inc(db1, 16)
            scalar.dma_start(xc_t[:, XB1h:XA1], xc_d[:, XB1h:XA1]).then_inc(dc1, 16)
            scalar.dma_start(vt_t[:, VH:], vt_d[:, VH:]).then_inc(drest, 16)
            scalar.dma_start(bt_t[:], bt_d[:]).then_inc(drest, 16)
            scalar.dma_start(wt_t[:, WSLOT:], wt_d[:, WSLOT:]).then_inc(dwt3, 16)
            scalar.dma_start(xa_t[:, XAr2:FLAT], xa_d[:, XAr2:FLAT]).then_inc(da2, 16)
            scalar.dma_start(xb_t[:, XAr2:FLAT], xb_d[:, XAr2:FLAT]).then_inc(db2, 16)
            scalar.dma_start(xc_t[:, XAr2:FLAT], xc_d[:, XAr2:FLAT]).then_inc(dc2, 16)
            # half 1 + final quarter of the last tile
            k = NT - 1
            lo = k * NFREE
            scalar.wait_ge(dve, NT)
            scalar.dma_start(out_d[:, lo:lo + NFREE // 2],
                             st_t[:, lo:lo + NFREE // 2]).then_inc(dout, 16)
            q3 = lo + NFREE // 2 + NFREE // 4
            scalar.wait_ge(dve, NT + 2)
            scalar.dma_start(out_d[:, q3:q3 + NFREE // 4],
                             st_t[:, q3:q3 + NFREE // 4]).then_inc(dout, 16)

        @block.tensor
        def _(tensor):
            # pre-warm the PE HAM clock gate during the initial DMA wait:
            # dummy matmuls (zeros into bank 7, which tile 7 later clears
            # with start=True) keep the PE busy from ~8.4us (preamble end)
            # until wave-1a lands (~12.5us), flipping the clock to 2.4GHz
            # right as the real stream begins. st_t is idle SBUF.
            tensor.wait_ge(warm, 1)
            for _ in range(5):
                nc.tensor.matmul(
                    ps_t[:, 7 * NFREE:8 * NFREE],
                    st_t[0:1, 0:128],
                    st_t[0:1, 0:NFREE],
                    start=True,
                    stop=True,
                )
            # phase A: xa-only accumulation for tiles 0..PHA-1, gated on the
            # just-in-time xa row chunks. Tile 0's first two slots gate only
            # on the micro-chunk so the real stream starts as early as the
            # very first weight bytes can physically arrive.
            tensor.wait_ge(dwt1, 16)
            tensor.wait_ge(da0, 16)
            for i, (s, view, kh, kw) in enumerate(slots_a):
                if i == 2:
                    tensor.wait_ge(dwt1, 32)
                nc.tensor.matmul(
                    ps_t[:, 0:NFREE],
                    wsl(s, 0),
                    view[:, kh: kh + RB, kw: kw + W],
                    start=(i == 0),
                    stop=False,
                )
            tensor.wait_ge(da0, 32)
            for k in range(1, 3):
                emit_mms(tensor, k, slots_a, start=True, stop=False)
            tensor.wait_ge(da1, 32)
            for k in range(3, PHA):
                emit_mms(tensor, k, slots_a, start=True, stop=False)
            # phase B: finish tiles 0..PHA-1 with the xb/xc pair slots
            tensor.wait_ge(dwt2, 16)
            tensor.wait_ge(db1, 32)
            tensor.wait_ge(dc1, 32)
            for k in range(PHA):
                emit_mms(tensor, k, slots_bc, start=False, stop=True)
            # steady state
            tensor.wait_ge(da2, 32)
            tensor.wait_ge(db2, 32)
            tensor.wait_ge(dc2, 32)
            # one bank-reuse wait covers 4 tiles: tiles k..k+3 need at most
            # dve >= k+3-(NPS-1) = k-4, and DVE lags PE by well under the
            # 3-tile slack this leaves. Fewer waits = fewer PE queue stalls.
            for k in range(PHA, NT - 1):
                if k == NBLK:
                    tensor.wait_ge(dwt3, 16)
                if (k - PHA) % 4 == 0:
                    tensor.wait_ge(dve, min(k + 3, NT - 1) - NPS + 1)
                emit_mms(tensor, k, slots_a, start=True, stop=False)
                emit_mms(tensor, k, slots_bc, start=False, stop=True)
            # final tile split into two 2-row groups (N=256 in half banks):
            # the first half's epilogue+DMA overlaps the second half's
            # matmuls, shortening the kernel tail
            k = NT - 1
            m, blk = divmod(k, NBLK)
            j0 = blk * RB
            for h in range(2):
                # halves in DIFFERENT banks (7, then 6): DVE reads half 1
                # while PE accumulates half 2, and same-bank PE-write +
                # DVE-read is a fatal PSUM collision. Bank 6 (tile 46) is
                # free once dve >= NT-1.
                if h == 1:
                    tensor.wait_ge(dve, NT - 1)
                ps_h = ps_t[:, (7 - h) * NFREE:(7 - h) * NFREE + NFREE // 2]
                for sl, is_last in ((slots_a, False), (slots_bc, True)):
                    n = len(sl)
                    for i, (s, view, kh, kw) in enumerate(sl):
                        mm = nc.tensor.matmul(
                            ps_h,
                            wsl(s, m),
                            view[:, j0 + 2 * h + kh: j0 + 2 * h + kh + RB // 2,
                                 kw: kw + W],
                            start=(sl is slots_a and i == 0),
                            stop=(is_last and i == n - 1),
                        )
                        if is_last and i == n - 1:
                            mm.then_inc(pes, 1)

        @block.vector
        def _(vector):
            nc.vector.memset(st_t[0:1, 0:NFREE], 0.0).then_inc(warm, 1)
            vector.wait_ge(drest, 48)  # bias + valid resident (3 chunks)
            for k in range(NT - 1):
                m, blk = divmod(k, NBLK)
                ps = ps_t[:, (k % NPS) * NFREE:(k % NPS + 1) * NFREE]
                vector.wait_ge(pes, k + 1)
                nc.vector.scalar_tensor_tensor(
                    st_t[:, k * NFREE:(k + 1) * NFREE],
                    ps,
                    bt_t[:, m:m + 1],
                    vt_t[:, blk * NFREE:(blk + 1) * NFREE],
                    mybir.AluOpType.add,
                    mybir.AluOpType.mult,
                ).then_inc(dve, 1)
            # final tile: half 1 in one epilogue, half 2 as two quarter
            # epilogues so the very last DVE->DMA gating unit is small
            k = NT - 1
            m, blk = divmod(k, NBLK)
            HF = NFREE // 2
            QF = NFREE // 4
            pieces = [(7, 0, HF), (6, HF, QF), (6, HF + QF, QF)]
            for bank, off, ln in pieces:
                ps_h = ps_t[:, bank * NFREE + (off - (HF if bank == 6 else 0)):
                            bank * NFREE + (off - (HF if bank == 6 else 0)) + ln]
                vector.wait_ge(pes, k + 1 + (1 if bank == 6 else 0))
                nc.vector.scalar_tensor_tensor(
                    st_t[:, k * NFREE + off:k * NFREE + off + ln],
                    ps_h,
                    bt_t[:, m:m + 1],
                    vt_t[:, blk * NFREE + off:blk * NFREE + off + ln],
                    mybir.AluOpType.add,
                    mybir.AluOpType.mult,
                ).then_inc(dve, 1)
    return nc


def _causal_mask():
    m = np.ones((KH, KW), dtype=np.float32)
    m[KH // 2, KW // 2:] = 0.0
    m[KH // 2 + 1:, :] = 0.0
    return m


def _prepare_in_maps(x, weight, bias, mask):
    # window-any of mask -> valid [B, H, W] float32
    ind = (np.asarray(mask)[:, 0] != 0)
    indp = np.zeros((B, H + 2 * PAD, W + 2 * PAD), dtype=bool)
    indp[:, PAD:PAD + H, PAD:PAD + W] = ind
    valid = np.zeros((B, H, W), dtype=bool)
    for dh in range(KH):
        for dw in range(KW):
            valid |= indp[:, dh:dh + H, dw:dw + W]
    valid_f = valid.astype(np.float32)

    w_bf = (np.asarray(weight, dtype=np.float32) * _causal_mask()[None, None]).astype(BF16)

    # 18 weight tiles [K=128, M=384], laid out m-major: [128, m, s, 128]
    wt = np.zeros((18, 128, COUT), dtype=BF16)
    for s, (kh, kw) in enumerate(TAPS):
        wt[s] = w_bf[:, 0:128, kh, kw].T
    for i, (ta, tb) in enumerate(PAIRS_XB):
        wt[12 + i, 0:64] = w_bf[:, 128:192, ta[0], ta[1]].T
        wt[12 + i, 64:128] = w_bf[:, 128:192, tb[0], tb[1]].T
    ta, tb = PAIR_XC
    wt[17, 0:64] = w_bf[:, 128:192, ta[0], ta[1]].T
    wt[17, 64:128] = w_bf[:, 128:192, tb[0], tb[1]].T
    # [18, 128, 3, 128] -> [128, 3, 18, 128]
    wt_sb = np.ascontiguousarray(
        wt.reshape(18, 128, 3, 128).transpose(1, 2, 0, 3))

    bias_t = np.ascontiguousarray(
        np.asarray(bias, dtype=np.float32).reshape(3, 128).T)

    x_bf = np.asarray(x, dtype=np.float32).astype(BF16)

    in_maps = []
    for c in range(NCORES):
        b, half = c // 2, c % 2
        r0 = half * HHALF
        xp = np.zeros((CIN, NROWS, WP), dtype=BF16)
        lo = r0 - PAD
        src_lo = max(lo, 0)
        xp[:, src_lo - lo:, PAD:PAD + W] = x_bf[b, :, src_lo:r0 + HHALF, :]
        xf = xp.reshape(CIN, FLAT)
        x2 = xf[128:192]
        sh1 = np.zeros_like(x2)
        sh1[:, :-1] = x2[:, 1:]
        shr = np.zeros_like(x2)
        shr[:, :-WP] = x2[:, WP:]
        vrow = valid_f[b, r0:r0 + HHALF].reshape(1, HHALF * W).astype(BF16)
        vt = np.ascontiguousarray(np.broadcast_to(vrow, (128, HHALF * W)))
        in_maps.append({
            "xa": np.ascontiguousarray(xf[0:128]),
            "xb": np.ascontiguousarray(np.concatenate([x2, sh1], axis=0)),
            "xc": np.ascontiguousarray(np.concatenate([x2, shr], axis=0)),
            "wt": wt_sb.reshape(128, 3 * 18 * 128),
            "bt": bias_t,
            "vt": vt,
        })
    return in_maps


def _assemble(results):
    out_full = np.zeros((B, COUT, H, W), dtype=np.float32)
    for c in range(NCORES):
        b, half = c // 2, c % 2
        o = np.asarray(results[c]["out"]).astype(np.float32)
        o4 = o.reshape(128, 3, HHALF, W).transpose(1, 0, 2, 3).reshape(COUT, HHALF, W)
        out_full[b, :, half * HHALF:(half + 1) * HHALF, :] = o4
    return out_full


def kernel(x, weight, bias, mask, _trace=False):
    in_maps = _prepare_in_maps(x, weight, bias, mask)
    nc = _build_program()
    res = run_bass_kernel_spmd(nc, in_maps, core_ids=list(range(NCORES)),
                               trace=_trace)
    out = _assemble(res.results)
    if _trace:
        return out, res
    return out
